# revision 11
# baseline (speedup 1.0000x reference)
"""AxialAttention Trainium2 Bass kernel (v3).

Problem: q,k,v of shape (4, 8, 16, 32, 32, 64) = (b, heads, t, h, w, d),
attention along the h axis (axis 3), softmax over keys, out same shape.

512 independent "slabs" (b, heads, t), each a batch of w=32 independent
length-32 attention problems with head dim 64.  64 slabs per NeuronCore
(8 cores), processed in "quads" (4 slabs).

Design (v3):
  - Q and K pre-transposed ON THE HOST to [slab, d, w, h]: loads land
    with d on partitions (no on-chip transposes), scores contract K=64
    in one matmul.
  - PAIRED scores matmuls: one matmul covers TWO w-columns with a
    [64, 64] stationary (k of w0 | k of w1) and [64, 64] moving
    (q of w0 | q of w1).  The off-diagonal cross-w blocks of the output
    are garbage but are simply never read downstream.  This halves the
    scores instruction count (PE instruction-fetch relief) and halves
    scores PE time (LDWEIGHTS rows == streamed rows == 64).
  - Scores for the 4 slabs of a quad land in two [128, (wh, p, q)]
    PSUM tiles; exp runs on all 128 partitions.
  - V is host-permuted to [g, s2, p, h, t2, wh, e] (e = d plus a baked
    ones column) so PV rhs partitions line up with the (s2, p) E-block
    rows; the ones column makes the softmax denominator fall out of the
    PV matmul.
  - Output written as bf16; normalize = one DVE tensor_mul reading PSUM
    and writing the bf16 output tile directly.
"""

import os
import sys
import numpy as np

for _p in ("/root/.axon_site/_ro/trn_rl_repo", "/opt/trn_rl_repo"):
    if os.path.isdir(_p) and _p not in sys.path:
        sys.path.append(_p)

B, NH, T, H, W, D = 4, 8, 16, 32, 32, 64
N_CORES = 8
NSLAB = B * NH * T  # 512
NSLAB_CORE = NSLAB // N_CORES  # 64
NQUAD = NSLAB_CORE // 4  # 16
VE = D + 1  # V row padded with a ones column for the denominator
WH = W // 2  # 16 w-pairs

_CACHED_NC = None


def _build_nc(n_slabs):
    import concourse.bacc as bacc
    import concourse.mybir as mybir
    from concourse import tile

    dt = mybir.dt
    nq = n_slabs // 4

    nc = bacc.Bacc("TRN2", target_bir_lowering=False, debug=False,
                   num_devices=N_CORES)
    qT_in = nc.dram_tensor("qT_in", [n_slabs, D, W, H], dt.bfloat16,
                           kind="ExternalInput").ap()
    kT_in = nc.dram_tensor("kT_in", [n_slabs, D, W, H], dt.bfloat16,
                           kind="ExternalInput").ap()
    v_in = nc.dram_tensor("v_in", [n_slabs, H, W, VE], dt.bfloat16,
                          kind="ExternalInput").ap()
    o_out = nc.dram_tensor("o_out", [n_slabs, H, W, D], dt.bfloat16,
                           kind="ExternalOutput").ap()

    scale = 1.0 / float(np.sqrt(D))

    with tile.TileContext(nc) as tc:
        with tc.tile_pool(name="io", bufs=2) as io_pool, \
             tc.tile_pool(name="vv", bufs=2) as v_pool, \
             tc.tile_pool(name="ee", bufs=4) as e_pool, \
             tc.tile_pool(name="oo", bufs=2) as o_pool, \
             tc.tile_pool(name="rr", bufs=4) as r_pool, \
             tc.tile_pool(name="ps_sc", bufs=2, space="PSUM") as ps_sc, \
             tc.tile_pool(name="ps_pv", bufs=6, space="PSUM") as ps_pv:

            quad_state = {}

            def emit_loads(g):
                s0 = 4 * g
                QA = io_pool.tile([128, W * H], dt.bfloat16, name="QA")
                QB = io_pool.tile([128, W * H], dt.bfloat16, name="QB")
                KA = io_pool.tile([128, W * H], dt.bfloat16, name="KA")
                KB = io_pool.tile([128, W * H], dt.bfloat16, name="KB")
                V4 = v_pool.tile([128, W, VE], dt.bfloat16, name="V4")
                nc.sync.dma_start(
                    out=QA[:, :],
                    in_=qT_in[s0:s0 + 2].rearrange("s d w h -> (s d) (w h)"))
                nc.sync.dma_start(
                    out=KA[:, :],
                    in_=kT_in[s0:s0 + 2].rearrange("s d w h -> (s d) (w h)"))
                nc.scalar.dma_start(
                    out=QB[:, :],
                    in_=qT_in[s0 + 2:s0 + 4].rearrange(
                        "s d w h -> (s d) (w h)"))
                nc.scalar.dma_start(
                    out=KB[:, :],
                    in_=kT_in[s0 + 2:s0 + 4].rearrange(
                        "s d w h -> (s d) (w h)"))
                nc.gpsimd.dma_start(
                    out=V4[:, :, :],
                    in_=v_in[s0:s0 + 4].rearrange("s h w e -> (s h) w e"))
                out_sb = o_pool.tile([128, W, D], dt.bfloat16, name="out_sb")
                quad_state[g] = dict(QA=QA, QB=QB, KA=KA, KB=KB, V4=V4,
                                     out_sb=out_sb)

            def emit_scores(g, chunk):
                # chunk covers 8 w; scores for all 4 slabs land in one
                # [128=(s,k), (w,q)] PSUM tile via column-packed matmuls.
                qs = quad_state[g]
                w0 = 8 * chunk
                psS = ps_sc.tile([128, 8, H], dt.float32, name="psS")
                for s in range(4):
                    QT = qs["QA"] if s < 2 else qs["QB"]
                    KT = qs["KA"] if s < 2 else qs["KB"]
                    ro = 64 * (s % 2)
                    for wl in range(8):
                        w = w0 + wl
                        nc.tensor.matmul(
                            psS[32 * s:32 * s + 32, wl, :],
                            lhsT=KT[ro:ro + 64, 32 * w:32 * w + 32],
                            rhs=QT[ro:ro + 64, 32 * w:32 * w + 32],
                            start=True, stop=True,
                            tile_position=(ro, 32 * s))
                E = e_pool.tile([128, 8, H], dt.bfloat16, name="E")
                nc.scalar.activation(
                    E[:, :, :].rearrange("p a b -> p (a b)"),
                    psS[:, :, :].rearrange("p a b -> p (a b)"),
                    mybir.ActivationFunctionType.Exp, scale=scale)
                return E

            def emit_pv(g, chunk, E):
                qs = quad_state[g]
                V4, out_sb = qs["V4"], qs["out_sb"]
                w0 = 8 * chunk
                for grp in range(2):
                    psPV = ps_pv.tile([128, 4, 128], dt.float32,
                                      name="psPV")
                    for s in range(4):
                        ro = 32 * s
                        for wl4 in range(4):
                            wl = 4 * grp + wl4
                            nc.tensor.matmul(
                                psPV[32 * s:32 * s + 32, wl4, 0:VE],
                                lhsT=E[ro:ro + 32, wl, :],
                                rhs=V4[ro:ro + 32, w0 + wl, :],
                                start=True, stop=True,
                                tile_position=(ro, ro))
                    R = r_pool.tile([128, 4], dt.float32, name="R")
                    nc.vector.reciprocal(R[:, :], psPV[:, :, D])
                    nc.vector.tensor_mul(
                        out_sb[:, w0 + 4 * grp:w0 + 4 * grp + 4, :],
                        psPV[:, :, 0:D],
                        R[:, :, None].broadcast_to([128, 4, D]))

            def emit_finish(g):
                qs = quad_state.pop(g)
                s0 = 4 * g
                eng = nc.sync if g % 2 == 0 else nc.scalar
                eng.dma_start(
                    out=o_out[s0:s0 + 4].rearrange("s h w d -> (s h) (w d)"),
                    in_=qs["out_sb"][:, :, :].rearrange("p w d -> p (w d)"))

            # Software pipeline: PV of chunk t is emitted after the scores
            # of chunk t+1 so the PE queue always has runnable matmuls.
            emit_loads(0)
            pending = None  # (g, chunk, E)
            for t in range(4 * nq):
                g, chunk = divmod(t, 4)
                if chunk == 0 and g + 1 < nq:
                    emit_loads(g + 1)
                E = emit_scores(g, chunk)
                if pending is not None:
                    pg, pc, pE = pending
                    emit_pv(pg, pc, pE)
                    if pc == 3:
                        emit_finish(pg)
                pending = (g, chunk, E)
            pg, pc, pE = pending
            emit_pv(pg, pc, pE)
            emit_finish(pg)
    nc.compile()
    return nc


def _get_nc():
    global _CACHED_NC
    if _CACHED_NC is None:
        _CACHED_NC = _build_nc(NSLAB_CORE)
    return _CACHED_NC


def kernel(q, k, v, decode_step=0, decode_idx=0, _trace=False):
    from concourse.bass_utils import run_bass_kernel_spmd

    import ml_dtypes
    bf16 = ml_dtypes.bfloat16
    q = np.asarray(q, dtype=np.float32).reshape(NSLAB, H, W, D).astype(bf16)
    k = np.asarray(k, dtype=np.float32).reshape(NSLAB, H, W, D).astype(bf16)
    v = np.asarray(v, dtype=np.float32).reshape(NSLAB, H, W, D).astype(bf16)
    qT = np.ascontiguousarray(q.transpose(0, 3, 2, 1))  # [slab, d, w, h]
    kT = np.ascontiguousarray(k.transpose(0, 3, 2, 1))
    vp = np.empty((NSLAB, H, W, VE), dtype=bf16)
    vp[..., :D] = v
    vp[..., D] = 1.0

    nc = _get_nc()
    in_maps = []
    nqc = NSLAB_CORE // 4
    for c in range(N_CORES):
        sl = slice(c * NSLAB_CORE, (c + 1) * NSLAB_CORE)
        in_maps.append({
            "qT_in": np.ascontiguousarray(qT[sl]),
            "kT_in": np.ascontiguousarray(kT[sl]),
            "v_in": np.ascontiguousarray(vp[sl]),
        })
    res = run_bass_kernel_spmd(nc, in_maps, core_ids=list(range(N_CORES)),
                               trace=_trace)
    out = np.concatenate([r["o_out"] for r in res.results], axis=0)
    out = out.reshape(B, NH, T, H, W, D).astype(np.float32)
    if _trace:
        return out, res
    return out


if __name__ == "__main__":
    rng = np.random.default_rng(0)
    shape = (B, NH, T, H, W, D)
    q = rng.standard_normal(shape, dtype=np.float32)
    k = rng.standard_normal(shape, dtype=np.float32)
    v = rng.standard_normal(shape, dtype=np.float32)
    out = kernel(q, k, v)
    print("kernel ran, out shape", out.shape)


# revision 12
# speedup vs baseline: 1.0519x; 1.0519x over previous
"""AxialAttention Trainium2 Bass kernel (v3).

Problem: q,k,v of shape (4, 8, 16, 32, 32, 64) = (b, heads, t, h, w, d),
attention along the h axis (axis 3), softmax over keys, out same shape.

512 independent "slabs" (b, heads, t), each a batch of w=32 independent
length-32 attention problems with head dim 64.  64 slabs per NeuronCore
(8 cores), processed in "quads" (4 slabs).

Design (v3):
  - Q and K pre-transposed ON THE HOST to [slab, d, w, h]: loads land
    with d on partitions (no on-chip transposes), scores contract K=64
    in one matmul.
  - PAIRED scores matmuls: one matmul covers TWO w-columns with a
    [64, 64] stationary (k of w0 | k of w1) and [64, 64] moving
    (q of w0 | q of w1).  The off-diagonal cross-w blocks of the output
    are garbage but are simply never read downstream.  This halves the
    scores instruction count (PE instruction-fetch relief) and halves
    scores PE time (LDWEIGHTS rows == streamed rows == 64).
  - Scores for the 4 slabs of a quad land in two [128, (wh, p, q)]
    PSUM tiles; exp runs on all 128 partitions.
  - V is host-permuted to [g, s2, p, h, t2, wh, e] (e = d plus a baked
    ones column) so PV rhs partitions line up with the (s2, p) E-block
    rows; the ones column makes the softmax denominator fall out of the
    PV matmul.
  - Output written as bf16; normalize = one DVE tensor_mul reading PSUM
    and writing the bf16 output tile directly.
"""

import os
import sys
import numpy as np

for _p in ("/root/.axon_site/_ro/trn_rl_repo", "/opt/trn_rl_repo"):
    if os.path.isdir(_p) and _p not in sys.path:
        sys.path.append(_p)

B, NH, T, H, W, D = 4, 8, 16, 32, 32, 64
N_CORES = 8
NSLAB = B * NH * T  # 512
NSLAB_CORE = NSLAB // N_CORES  # 64
NQUAD = NSLAB_CORE // 4  # 16
VE = D + 1  # V row padded with a ones column for the denominator
WH = W // 2  # 16 w-pairs

_CACHED_NC = None


def _build_nc(n_slabs):
    import concourse.bacc as bacc
    import concourse.mybir as mybir
    from concourse import tile

    dt = mybir.dt
    nq = n_slabs // 4

    nc = bacc.Bacc("TRN2", target_bir_lowering=False, debug=False,
                   num_devices=N_CORES)
    qT_in = nc.dram_tensor("qT_in", [n_slabs, D, W, H], dt.bfloat16,
                           kind="ExternalInput").ap()
    kT_in = nc.dram_tensor("kT_in", [n_slabs, D, W, H], dt.bfloat16,
                           kind="ExternalInput").ap()
    v_in = nc.dram_tensor("v_in", [n_slabs, H, W, VE], dt.bfloat16,
                          kind="ExternalInput").ap()
    o_out = nc.dram_tensor("o_out", [n_slabs, H, W, D], dt.bfloat16,
                           kind="ExternalOutput").ap()

    scale = 1.0 / float(np.sqrt(D))

    with tile.TileContext(nc) as tc:
        with tc.tile_pool(name="io", bufs=2) as io_pool, \
             tc.tile_pool(name="vv", bufs=2) as v_pool, \
             tc.tile_pool(name="ee", bufs=2) as e_pool, \
             tc.tile_pool(name="oo", bufs=2) as o_pool, \
             tc.tile_pool(name="rr", bufs=4) as r_pool, \
             tc.tile_pool(name="ps_sc", bufs=2, space="PSUM") as ps_sc, \
             tc.tile_pool(name="ps_pv", bufs=6, space="PSUM") as ps_pv:

            quad_state = {}

            def emit_loads(g):
                s0 = 4 * g
                QA = io_pool.tile([128, W * H], dt.bfloat16, name="QA")
                QB = io_pool.tile([128, W * H], dt.bfloat16, name="QB")
                KA = io_pool.tile([128, W * H], dt.bfloat16, name="KA")
                KB = io_pool.tile([128, W * H], dt.bfloat16, name="KB")
                V4 = v_pool.tile([128, W, VE], dt.bfloat16, name="V4")
                nc.sync.dma_start(
                    out=QA[:, :],
                    in_=qT_in[s0:s0 + 2].rearrange("s d w h -> (s d) (w h)"))
                nc.sync.dma_start(
                    out=KA[:, :],
                    in_=kT_in[s0:s0 + 2].rearrange("s d w h -> (s d) (w h)"))
                nc.scalar.dma_start(
                    out=QB[:, :],
                    in_=qT_in[s0 + 2:s0 + 4].rearrange(
                        "s d w h -> (s d) (w h)"))
                nc.scalar.dma_start(
                    out=KB[:, :],
                    in_=kT_in[s0 + 2:s0 + 4].rearrange(
                        "s d w h -> (s d) (w h)"))
                nc.gpsimd.dma_start(
                    out=V4[:, :, :],
                    in_=v_in[s0:s0 + 4].rearrange("s h w e -> (s h) w e"))
                out_sb = o_pool.tile([128, W, D], dt.bfloat16, name="out_sb")
                quad_state[g] = dict(QA=QA, QB=QB, KA=KA, KB=KB, V4=V4,
                                     out_sb=out_sb)

            def emit_scores(g, chunk):
                # chunk covers 16 w; scores for all 4 slabs land in one
                # [128=(s,k), (w,q)] PSUM tile via column-packed matmuls.
                qs = quad_state[g]
                w0 = 16 * chunk
                psS = ps_sc.tile([128, 16, H], dt.float32, name="psS")
                for s in range(4):
                    QT = qs["QA"] if s < 2 else qs["QB"]
                    KT = qs["KA"] if s < 2 else qs["KB"]
                    ro = 64 * (s % 2)
                    for wl in range(16):
                        w = w0 + wl
                        nc.tensor.matmul(
                            psS[32 * s:32 * s + 32, wl, :],
                            lhsT=KT[ro:ro + 64, 32 * w:32 * w + 32],
                            rhs=QT[ro:ro + 64, 32 * w:32 * w + 32],
                            start=True, stop=True,
                            tile_position=(ro, 32 * s))
                E = e_pool.tile([128, 16, H], dt.bfloat16, name="E")
                nc.scalar.activation(
                    E[:, :, :].rearrange("p a b -> p (a b)"),
                    psS[:, :, :].rearrange("p a b -> p (a b)"),
                    mybir.ActivationFunctionType.Exp, scale=scale)
                return E

            def emit_pv(g, chunk, E):
                qs = quad_state[g]
                V4, out_sb = qs["V4"], qs["out_sb"]
                w0 = 16 * chunk
                for grp in range(4):
                    psPV = ps_pv.tile([128, 4, 128], dt.float32,
                                      name="psPV")
                    for s in range(4):
                        ro = 32 * s
                        for wl4 in range(4):
                            wl = 4 * grp + wl4
                            nc.tensor.matmul(
                                psPV[32 * s:32 * s + 32, wl4, 0:VE],
                                lhsT=E[ro:ro + 32, wl, :],
                                rhs=V4[ro:ro + 32, w0 + wl, :],
                                start=True, stop=True,
                                tile_position=(ro, ro))
                    R = r_pool.tile([128, 4], dt.float32, name="R")
                    nc.vector.reciprocal(R[:, :], psPV[:, :, D])
                    nc.vector.tensor_mul(
                        out_sb[:, w0 + 4 * grp:w0 + 4 * grp + 4, :],
                        psPV[:, :, 0:D],
                        R[:, :, None].broadcast_to([128, 4, D]))

            def emit_finish(g):
                qs = quad_state.pop(g)
                s0 = 4 * g
                nc.sync.dma_start(
                    out=o_out[s0:s0 + 4].rearrange("s h w d -> (s h) (w d)"),
                    in_=qs["out_sb"][:, :, :].rearrange("p w d -> p (w d)"))

            # Software pipeline: PV of chunk t is emitted after the scores
            # of chunk t+1 so the PE queue always has runnable matmuls.
            emit_loads(0)
            pending = None  # (g, chunk, Es)
            for t in range(2 * nq):
                g, chunk = divmod(t, 2)
                if chunk == 0 and g + 1 < nq:
                    emit_loads(g + 1)
                E = emit_scores(g, chunk)
                if pending is not None:
                    pg, pc, pE = pending
                    emit_pv(pg, pc, pE)
                    if pc == 1:
                        emit_finish(pg)
                pending = (g, chunk, E)
            pg, pc, pE = pending
            emit_pv(pg, pc, pE)
            emit_finish(pg)
    nc.compile()
    return nc


def _get_nc():
    global _CACHED_NC
    if _CACHED_NC is None:
        _CACHED_NC = _build_nc(NSLAB_CORE)
    return _CACHED_NC


def kernel(q, k, v, decode_step=0, decode_idx=0, _trace=False):
    from concourse.bass_utils import run_bass_kernel_spmd

    import ml_dtypes
    bf16 = ml_dtypes.bfloat16
    q = np.asarray(q, dtype=np.float32).reshape(NSLAB, H, W, D).astype(bf16)
    k = np.asarray(k, dtype=np.float32).reshape(NSLAB, H, W, D).astype(bf16)
    v = np.asarray(v, dtype=np.float32).reshape(NSLAB, H, W, D).astype(bf16)
    qT = np.ascontiguousarray(q.transpose(0, 3, 2, 1))  # [slab, d, w, h]
    kT = np.ascontiguousarray(k.transpose(0, 3, 2, 1))
    vp = np.empty((NSLAB, H, W, VE), dtype=bf16)
    vp[..., :D] = v
    vp[..., D] = 1.0

    nc = _get_nc()
    in_maps = []
    nqc = NSLAB_CORE // 4
    for c in range(N_CORES):
        sl = slice(c * NSLAB_CORE, (c + 1) * NSLAB_CORE)
        in_maps.append({
            "qT_in": np.ascontiguousarray(qT[sl]),
            "kT_in": np.ascontiguousarray(kT[sl]),
            "v_in": np.ascontiguousarray(vp[sl]),
        })
    res = run_bass_kernel_spmd(nc, in_maps, core_ids=list(range(N_CORES)),
                               trace=_trace)
    out = np.concatenate([r["o_out"] for r in res.results], axis=0)
    out = out.reshape(B, NH, T, H, W, D).astype(np.float32)
    if _trace:
        return out, res
    return out


if __name__ == "__main__":
    rng = np.random.default_rng(0)
    shape = (B, NH, T, H, W, D)
    q = rng.standard_normal(shape, dtype=np.float32)
    k = rng.standard_normal(shape, dtype=np.float32)
    v = rng.standard_normal(shape, dtype=np.float32)
    out = kernel(q, k, v)
    print("kernel ran, out shape", out.shape)
